# revision 28
# baseline (speedup 1.0000x reference)
"""Multi-head attention Trainium2 kernel (B=4, S=2048, E=1024, H=16).

Sharding: 8 cores = 4 batch groups x 2-way head tensor-parallel.
Core c handles batch b=c//2 and heads [g*8, g*8+8) with g=c%2.
Each core computes its partial output projection; a 2-way ReduceScatter
pair-sums the partials in 8 row chunks, so core c ends with interleaved
128-row slabs of batch b's final output. The host assembles the full
[4,2048,1024] result.

Device layout notes:
- x arrives pre-transposed from the host as xT[e,s] so every matmul
  contracts over the SBUF partition dim.
- Q,K are produced transposed (QT/KT [d,s]) with head pairs stacked on the
  128 partitions; scores are computed transposed (scoresT[k,q]) so the exp
  output PT[k,q] feeds the P@V matmul directly as the moving operand.
- exp runs on 1024-wide strips (two query blocks) to amortize the ACT
  engine's ~293-cycle per-op overhead; P@V runs one key-block behind the
  scores stream so the PE never waits on the exp it just scheduled.
- V gets a ones-column (65th) so the P@V matmul also emits the softmax
  denominator row for free; normalization uses a fast DVE reciprocal.
- All matmuls run in bf16 (fp32 PSUM accumulate).
"""

import os
import sys

import numpy as np

for _p in ("/opt/trn_rl_repo", "/root/.axon_site/_ro/trn_rl_repo"):
    if os.path.isdir(_p) and _p not in sys.path:
        sys.path.append(_p)

import ml_dtypes  # noqa: E402
from concourse import bacc, mybir, tile  # noqa: E402
from concourse.bass_utils import run_bass_kernel_spmd  # noqa: E402

B, S, E, H, DH = 4, 2048, 1024, 16, 64
N_CORES = 8
TP = 2  # head-parallel factor within a batch
H_LOC = H // TP  # 8 heads per core
EI_LOC = H_LOC * DH  # 512 local rows of the concat dim
N_SB = S // 128  # 16 token blocks
N_EC = E // 128  # 8 contraction chunks
N_QB = S // 512  # 4 query blocks
N_KB = S // 128  # 16 key blocks
N_HP = H_LOC // 2  # 4 head pairs
# ReduceScatter chunk schedule: (row0, nrows, emit_after_qb)
RS_CHUNKS = [(0, 512, 0), (512, 512, 1), (1024, 512, 2), (1536, 512, 3)]

BF = mybir.dt.bfloat16
F32 = mybir.dt.float32
EXP = mybir.ActivationFunctionType.Exp
MULT = mybir.AluOpType.mult

_CACHE = {}


def _build():
    nc = bacc.Bacc("TRN2", target_bir_lowering=False, debug=False,
                   num_devices=N_CORES)

    xT_in = nc.declare_dram_parameter("xT", [E, S], BF, isOutput=False)
    wq_in = nc.declare_dram_parameter("wq", [E, EI_LOC], BF, isOutput=False)
    wk_in = nc.declare_dram_parameter("wk", [E, EI_LOC], BF, isOutput=False)
    wv_in = nc.declare_dram_parameter("wv", [E, EI_LOC], BF, isOutput=False)
    woT_in = nc.declare_dram_parameter("woT", [EI_LOC, E], BF, isOutput=False)
    bob_in = nc.declare_dram_parameter("bob", [128, E], F32, isOutput=False)
    y_out = nc.declare_dram_parameter("y", [S // TP, E], F32, isOutput=True)

    y_part = nc.dram_tensor("y_part", [S, E], F32)
    y_chunks = [nc.dram_tensor(f"y_chunk{i}", [n // 2, E], F32)
                for i, (_, n, _) in enumerate(RS_CHUNKS)]

    inv_sqrt_dh = 1.0 / float(np.sqrt(DH))

    with tile.TileContext(nc) as tc:
        with (
            tc.tile_pool(name="const", bufs=1) as constp,
            tc.tile_pool(name="persist", bufs=1) as persist,
        ):
            # ---- input DMAs (xT first: projections are the startup
            # critical path) ----
            xTp = tc.alloc_tile_pool(name="xTp", bufs=1)
            xT = [xTp.tile([128, S], BF, tag=f"xT{ec}", name=f"xT{ec}")
                  for ec in range(N_EC)]
            for q in range(4):
                cs = slice(q * 512, (q + 1) * 512)
                for ec in range(N_EC):
                    nc.sync.dma_start(
                        xT[ec][:, cs], xT_in[ec * 128:(ec + 1) * 128, cs])
            wq_t, wk_t, wv_t = [], [], []
            for ec in range(N_EC):
                for lst, src, nm in ((wv_t, wv_in, "wv"), (wq_t, wq_in, "wq"),
                                     (wk_t, wk_in, "wk")):
                    t = constp.tile([128, EI_LOC], BF, tag=f"{nm}{ec}",
                                    name=f"{nm}{ec}")
                    nc.sync.dma_start(t[:], src[ec * 128:(ec + 1) * 128, :])
                    lst.append(t)
            woT_t = []
            for c in range(4):
                t = constp.tile([128, E], BF, tag=f"woT{c}", name=f"woT{c}")
                nc.sync.dma_start(t[:], woT_in[c * 128:(c + 1) * 128, :])
                woT_t.append(t)
            bob = constp.tile([128, E], F32, tag="bob")
            nc.sync.dma_start(bob[:], bob_in[:])

            # Per-head QT/KT tiles zero-padded to K=128 so the scores
            # matmuls stream the full PE array (keeps the activity monitor
            # out of its throttled state; data rows 0-63, zeros 64-127).
            QT = [persist.tile([128, S], BF, tag=f"QT{h}", name=f"QT{h}")
                  for h in range(H_LOC)]
            KT = [persist.tile([128, S], BF, tag=f"KT{h}", name=f"KT{h}")
                  for h in range(H_LOC)]
            for h in range(H_LOC):
                nc.vector.memset(QT[h][64:128, :], 0.0)
                nc.vector.memset(KT[h][64:128, :], 0.0)
            V = [persist.tile([128, H_LOC, DH + 1], BF, tag=f"V{s}",
                              name=f"V{s}") for s in range(N_SB)]
            CT = [persist.tile([128, S], BF, tag=f"CT{c}", name=f"CT{c}")
                  for c in range(4)]

            # ---- phase A: projections (own PSUM pool, closed before
            # attention) ----
            with tc.tile_pool(name="projps", bufs=4, space="PSUM") as projps:
                # V projection (natural layout [s, h*d]) + ones column
                for sb in range(N_SB):
                    ps = projps.tile([128, EI_LOC], F32, tag="projp",
                                     name="vps")
                    for ec in range(N_EC):
                        nc.tensor.matmul(
                            ps[:], xT[ec][:, sb * 128:(sb + 1) * 128],
                            wv_t[ec][:], start=(ec == 0),
                            stop=(ec == N_EC - 1))
                    nc.vector.tensor_copy(V[sb][:, :, 0:DH], ps[:])
                    nc.vector.memset(V[sb][:, :, DH], 1.0)

                # Q/K transposed projections, head pairs on partitions
                for hp in range(N_HP):
                    for qb in range(N_QB):
                        for dst, w in ((QT, wq_t), (KT, wk_t)):
                            ps = projps.tile([128, 512], F32, tag="projp",
                                             name="qkps")
                            for ec in range(N_EC):
                                nc.tensor.matmul(
                                    ps[:],
                                    w[ec][:, hp * 128:(hp + 1) * 128],
                                    xT[ec][:, qb * 512:(qb + 1) * 512],
                                    start=(ec == 0), stop=(ec == N_EC - 1))
                            cols = slice(qb * 512, (qb + 1) * 512)
                            nc.vector.tensor_copy(
                                dst[2 * hp][0:64, cols], ps[0:64, :])
                            nc.vector.tensor_copy(
                                dst[2 * hp + 1][0:64, cols], ps[64:128, :])

            xTp.release()

            # ---- phase B: attention ----
            with (
                tc.tile_pool(name="scps", bufs=2, space="PSUM") as scps,
                tc.tile_pool(name="pvps", bufs=4, space="PSUM") as pvps,
                tc.tile_pool(name="ptp", bufs=3) as ptp,
                tc.tile_pool(name="smallp", bufs=3) as smallp,
                tc.tile_pool(name="youtp", bufs=3) as youtp,
            ):
                _attention(nc, tc, scps, pvps, ptp, smallp, youtp,
                           QT, KT, V, CT, woT_t, bob,
                           y_part, y_chunks, y_out, inv_sqrt_dh)

    nc.finalize()
    return nc


def _attention(nc, tc, scps, pvps, ptp, smallp, youtp, QT, KT, V, CT, woT_t,
               bob, y_part, y_chunks, y_out, inv_sqrt_dh):
    if True:  # keep indentation shallow
        if True:
            for qp in range(N_QB // 2):  # query pair-blocks of 1024
                for h in range(H_LOC):
                    hp, hh = h // 2, h % 2
                    rows = slice(hh * 64, (hh + 1) * 64)
                    pv0 = pvps.tile([DH + 1, 512], F32, tag="pv", name="pv0")
                    pv1 = pvps.tile([DH + 1, 512], F32, tag="pv", name="pv1")
                    prev_pt = None
                    for kb in range(N_KB):
                        sp = scps.tile([128, 1024], F32, tag="sc", name="sc")
                        for half in range(2):
                            q5 = slice((2 * qp + half) * 512,
                                       (2 * qp + half + 1) * 512)
                            nc.tensor.matmul(
                                sp[:, half * 512:(half + 1) * 512],
                                KT[h][:, kb * 128:(kb + 1) * 128],
                                QT[h][:, q5])
                        pt = ptp.tile([128, 1024], BF, tag="pt", name="pt")
                        nc.scalar.activation(pt[:], sp[:], EXP,
                                             scale=inv_sqrt_dh)
                        if prev_pt is not None:
                            pkb = kb - 1
                            nc.tensor.matmul(
                                pv0[:], V[pkb][:, h, :], prev_pt[:, 0:512],
                                start=(pkb == 0), stop=False)
                            nc.tensor.matmul(
                                pv1[:], V[pkb][:, h, :], prev_pt[:, 512:1024],
                                start=(pkb == 0), stop=False)
                        prev_pt = pt
                    nc.tensor.matmul(pv0[:], V[N_KB - 1][:, h, :],
                                     prev_pt[:, 0:512],
                                     start=False, stop=True)
                    nc.tensor.matmul(pv1[:], V[N_KB - 1][:, h, :],
                                     prev_pt[:, 512:1024],
                                     start=False, stop=True)

                    for half, pv in ((0, pv0), (1, pv1)):
                        qs = slice((2 * qp + half) * 512,
                                   (2 * qp + half + 1) * 512)
                        den = smallp.tile([1, 512], F32, tag="den",
                                          name="den")
                        nc.vector.tensor_copy(den[:], pv[DH:DH + 1, :])
                        denb = smallp.tile([64, 512], F32, tag="denb",
                                           name="denb")
                        nc.gpsimd.partition_broadcast(denb[:], den[:])
                        rec = smallp.tile([64, 512], F32, tag="rec",
                                          name="rec")
                        nc.vector.reciprocal_approx_fast(rec[:], denb[:])
                        nc.vector.tensor_tensor(
                            CT[hp][rows, qs], pv[0:DH, :], rec[:], MULT)

                # output projection + chunked ReduceScatter
                for qb in (2 * qp, 2 * qp + 1):
                    for sb in range(4 * qb, 4 * qb + 4):
                        ys = scps.tile([128, 1024], F32, tag="sc", name="ys")
                        for eo in range(2):
                            for c in range(4):
                                nc.tensor.matmul(
                                    ys[:, eo * 512:(eo + 1) * 512],
                                    CT[c][:, sb * 128:(sb + 1) * 128],
                                    woT_t[c][:, eo * 512:(eo + 1) * 512],
                                    start=(c == 0), stop=(c == 3))
                        yt = youtp.tile([128, E], F32, tag="yt", name="yt")
                        nc.vector.tensor_add(yt[:], ys[:], bob[:])
                        nc.sync.dma_start(
                            y_part[sb * 128:(sb + 1) * 128, :], yt[:])
                    for i, (r0, n, after) in enumerate(RS_CHUNKS):
                        if after != qb:
                            continue
                        nc.gpsimd.collective_compute(
                            "ReduceScatter", mybir.AluOpType.add,
                            replica_groups=[[0, 1], [2, 3], [4, 5], [6, 7]],
                            ins=[y_part[r0:r0 + n, :]],
                            outs=[y_chunks[i][:]])
                        nc.sync.dma_start(
                            y_out[r0 // 2:(r0 + n) // 2, :], y_chunks[i][:])


def _get_nc():
    if "nc" not in _CACHE:
        _CACHE["nc"] = _build()
    return _CACHE["nc"]


def _make_in_maps(x, wq, wk, wv, wo, bo):
    bf16 = ml_dtypes.bfloat16
    x, wq, wk, wv, wo, bo = (np.asarray(a) for a in (x, wq, wk, wv, wo, bo))
    in_maps = []
    for c in range(N_CORES):
        b, g = c // TP, c % TP
        h0 = g * H_LOC
        xT_l = np.ascontiguousarray(x[b].T).astype(bf16)
        wq_l = np.ascontiguousarray(
            wq[h0:h0 + H_LOC].transpose(1, 0, 2).reshape(E, EI_LOC)).astype(bf16)
        wk_l = np.ascontiguousarray(
            wk[h0:h0 + H_LOC].transpose(1, 0, 2).reshape(E, EI_LOC)).astype(bf16)
        wv_l = np.ascontiguousarray(
            wv[h0:h0 + H_LOC].transpose(1, 0, 2).reshape(E, EI_LOC)).astype(bf16)
        woT_l = np.ascontiguousarray(
            wo[:, g * EI_LOC:(g + 1) * EI_LOC].T).astype(bf16)
        bob = np.broadcast_to(bo.astype(np.float32) / TP, (128, E)).copy()
        in_maps.append({
            "xT": xT_l, "wq": wq_l, "wk": wk_l, "wv": wv_l, "woT": woT_l,
            "bob": bob,
        })
    return in_maps


def _assemble(results):
    out = np.empty((B, S, E), dtype=np.float32)
    for c in range(N_CORES):
        b, g = c // TP, c % TP
        y = results[c]["y"]
        for r0, n, _ in RS_CHUNKS:
            half = n // 2
            out[b, r0 + g * half:r0 + (g + 1) * half, :] =                 y[r0 // 2:r0 // 2 + half, :]
    return out


def kernel(x, wq, wk, wv, wo, bo):
    nc = _get_nc()
    in_maps = _make_in_maps(x, wq, wk, wv, wo, bo)
    res = run_bass_kernel_spmd(nc, in_maps, list(range(N_CORES)))
    return _assemble(res.results)


# revision 29
# speedup vs baseline: 1.0363x; 1.0363x over previous
"""Multi-head attention Trainium2 kernel (B=4, S=2048, E=1024, H=16).

Sharding: 8 cores = 4 batch groups x 2-way head tensor-parallel.
Core c handles batch b=c//2 and heads [g*8, g*8+8) with g=c%2.
Each core computes its partial output projection; a 2-way ReduceScatter
pair-sums the partials in 8 row chunks, so core c ends with interleaved
128-row slabs of batch b's final output. The host assembles the full
[4,2048,1024] result.

Device layout notes:
- x arrives pre-transposed from the host as xT[e,s] so every matmul
  contracts over the SBUF partition dim.
- Q,K are produced transposed (QT/KT [d,s]) with head pairs stacked on the
  128 partitions; scores are computed transposed (scoresT[k,q]) so the exp
  output PT[k,q] feeds the P@V matmul directly as the moving operand.
- exp runs on 1024-wide strips (two query blocks) to amortize the ACT
  engine's ~293-cycle per-op overhead; P@V runs one key-block behind the
  scores stream so the PE never waits on the exp it just scheduled.
- V gets a ones-column (65th) so the P@V matmul also emits the softmax
  denominator row for free; normalization uses a fast DVE reciprocal.
- All matmuls run in bf16 (fp32 PSUM accumulate).
"""

import os
import sys

import numpy as np

for _p in ("/opt/trn_rl_repo", "/root/.axon_site/_ro/trn_rl_repo"):
    if os.path.isdir(_p) and _p not in sys.path:
        sys.path.append(_p)

import ml_dtypes  # noqa: E402
from concourse import bacc, mybir, tile  # noqa: E402
from concourse.bass_utils import run_bass_kernel_spmd  # noqa: E402

B, S, E, H, DH = 4, 2048, 1024, 16, 64
N_CORES = 8
TP = 2  # head-parallel factor within a batch
H_LOC = H // TP  # 8 heads per core
EI_LOC = H_LOC * DH  # 512 local rows of the concat dim
N_SB = S // 128  # 16 token blocks
N_EC = E // 128  # 8 contraction chunks
N_QB = S // 512  # 4 query blocks
N_KB = S // 128  # 16 key blocks
N_HP = H_LOC // 2  # 4 head pairs
# ReduceScatter chunk schedule: (row0, nrows, emit_after_qb)
RS_CHUNKS = [(0, 1024, 1), (1024, 1024, 3)]

BF = mybir.dt.bfloat16
F32 = mybir.dt.float32
EXP = mybir.ActivationFunctionType.Exp
MULT = mybir.AluOpType.mult

_CACHE = {}


def _build():
    nc = bacc.Bacc("TRN2", target_bir_lowering=False, debug=False,
                   num_devices=N_CORES)

    xT_in = nc.declare_dram_parameter("xT", [E, S], BF, isOutput=False)
    wq_in = nc.declare_dram_parameter("wq", [E, EI_LOC], BF, isOutput=False)
    wk_in = nc.declare_dram_parameter("wk", [E, EI_LOC], BF, isOutput=False)
    wv_in = nc.declare_dram_parameter("wv", [E, EI_LOC], BF, isOutput=False)
    woT_in = nc.declare_dram_parameter("woT", [EI_LOC, E], BF, isOutput=False)
    bob_in = nc.declare_dram_parameter("bob", [128, E], F32, isOutput=False)
    y_out = nc.declare_dram_parameter("y", [S // TP, E], F32, isOutput=True)

    y_part = nc.dram_tensor("y_part", [S, E], F32)
    y_chunks = [nc.dram_tensor(f"y_chunk{i}", [n // 2, E], F32)
                for i, (_, n, _) in enumerate(RS_CHUNKS)]

    inv_sqrt_dh = 1.0 / float(np.sqrt(DH))

    with tile.TileContext(nc) as tc:
        with (
            tc.tile_pool(name="const", bufs=1) as constp,
            tc.tile_pool(name="persist", bufs=1) as persist,
        ):
            # ---- input DMAs (xT first: projections are the startup
            # critical path) ----
            xTp = tc.alloc_tile_pool(name="xTp", bufs=1)
            xT = [xTp.tile([128, S], BF, tag=f"xT{ec}", name=f"xT{ec}")
                  for ec in range(N_EC)]
            cs0 = slice(0, 512)
            for ec in range(N_EC):
                nc.sync.dma_start(
                    xT[ec][:, cs0], xT_in[ec * 128:(ec + 1) * 128, cs0])
            wq_t, wk_t, wv_t = [], [], []
            for ec in range(N_EC):
                t = constp.tile([128, EI_LOC], BF, tag=f"wv{ec}",
                                name=f"wv{ec}")
                nc.sync.dma_start(t[:], wv_in[ec * 128:(ec + 1) * 128, :])
                wv_t.append(t)
            for q in range(1, 4):
                cs = slice(q * 512, (q + 1) * 512)
                for ec in range(N_EC):
                    nc.sync.dma_start(
                        xT[ec][:, cs], xT_in[ec * 128:(ec + 1) * 128, cs])
            for ec in range(N_EC):
                for lst, src, nm in ((wq_t, wq_in, "wq"), (wk_t, wk_in, "wk")):
                    t = constp.tile([128, EI_LOC], BF, tag=f"{nm}{ec}",
                                    name=f"{nm}{ec}")
                    nc.sync.dma_start(t[:], src[ec * 128:(ec + 1) * 128, :])
                    lst.append(t)
            woT_t = []
            for c in range(4):
                t = constp.tile([128, E], BF, tag=f"woT{c}", name=f"woT{c}")
                nc.sync.dma_start(t[:], woT_in[c * 128:(c + 1) * 128, :])
                woT_t.append(t)
            bob = constp.tile([128, E], F32, tag="bob")
            nc.sync.dma_start(bob[:], bob_in[:])

            # Per-head QT/KT tiles zero-padded to K=128 so the scores
            # matmuls stream the full PE array (keeps the activity monitor
            # out of its throttled state; data rows 0-63, zeros 64-127).
            QT = [persist.tile([128, S], BF, tag=f"QT{h}", name=f"QT{h}")
                  for h in range(H_LOC)]
            KT = [persist.tile([128, S], BF, tag=f"KT{h}", name=f"KT{h}")
                  for h in range(H_LOC)]
            for h in range(H_LOC):
                nc.vector.memset(QT[h][64:128, :], 0.0)
                nc.vector.memset(KT[h][64:128, :], 0.0)
            V = [persist.tile([128, H_LOC, DH + 1], BF, tag=f"V{s}",
                              name=f"V{s}") for s in range(N_SB)]
            CT = [persist.tile([128, S], BF, tag=f"CT{c}", name=f"CT{c}")
                  for c in range(4)]

            # ---- phase A: projections (own PSUM pool, closed before
            # attention) ----
            with tc.tile_pool(name="projps", bufs=4, space="PSUM") as projps:
                # V projection (natural layout [s, h*d]) + ones column
                for sb in range(N_SB):
                    ps = projps.tile([128, EI_LOC], F32, tag="projp",
                                     name="vps")
                    for ec in range(N_EC):
                        nc.tensor.matmul(
                            ps[:], xT[ec][:, sb * 128:(sb + 1) * 128],
                            wv_t[ec][:], start=(ec == 0),
                            stop=(ec == N_EC - 1))
                    nc.vector.tensor_copy(V[sb][:, :, 0:DH], ps[:])
                    nc.vector.memset(V[sb][:, :, DH], 1.0)

                # Q/K transposed projections, head pairs on partitions
                for hp in range(N_HP):
                    for qb in range(N_QB):
                        for dst, w in ((QT, wq_t), (KT, wk_t)):
                            ps = projps.tile([128, 512], F32, tag="projp",
                                             name="qkps")
                            for ec in range(N_EC):
                                nc.tensor.matmul(
                                    ps[:],
                                    w[ec][:, hp * 128:(hp + 1) * 128],
                                    xT[ec][:, qb * 512:(qb + 1) * 512],
                                    start=(ec == 0), stop=(ec == N_EC - 1))
                            cols = slice(qb * 512, (qb + 1) * 512)
                            nc.vector.tensor_copy(
                                dst[2 * hp][0:64, cols], ps[0:64, :])
                            nc.vector.tensor_copy(
                                dst[2 * hp + 1][0:64, cols], ps[64:128, :])

            xTp.release()

            # ---- phase B: attention ----
            with (
                tc.tile_pool(name="scps", bufs=2, space="PSUM") as scps,
                tc.tile_pool(name="pvps", bufs=4, space="PSUM") as pvps,
                tc.tile_pool(name="ptp", bufs=3) as ptp,
                tc.tile_pool(name="smallp", bufs=3) as smallp,
                tc.tile_pool(name="youtp", bufs=3) as youtp,
            ):
                _attention(nc, tc, scps, pvps, ptp, smallp, youtp,
                           QT, KT, V, CT, woT_t, bob,
                           y_part, y_chunks, y_out, inv_sqrt_dh)

    nc.finalize()
    return nc


def _attention(nc, tc, scps, pvps, ptp, smallp, youtp, QT, KT, V, CT, woT_t,
               bob, y_part, y_chunks, y_out, inv_sqrt_dh):
    if True:  # keep indentation shallow
        if True:
            for qp in range(N_QB // 2):  # query pair-blocks of 1024
                for h in range(H_LOC):
                    hp, hh = h // 2, h % 2
                    rows = slice(hh * 64, (hh + 1) * 64)
                    pv0 = pvps.tile([DH + 1, 512], F32, tag="pv", name="pv0")
                    pv1 = pvps.tile([DH + 1, 512], F32, tag="pv", name="pv1")
                    prev_pt = None
                    for kb in range(N_KB):
                        sp = scps.tile([128, 1024], F32, tag="sc", name="sc")
                        for half in range(2):
                            q5 = slice((2 * qp + half) * 512,
                                       (2 * qp + half + 1) * 512)
                            nc.tensor.matmul(
                                sp[:, half * 512:(half + 1) * 512],
                                KT[h][:, kb * 128:(kb + 1) * 128],
                                QT[h][:, q5])
                        pt = ptp.tile([128, 1024], BF, tag="pt", name="pt")
                        nc.scalar.activation(pt[:], sp[:], EXP,
                                             scale=inv_sqrt_dh)
                        if prev_pt is not None:
                            pkb = kb - 1
                            nc.tensor.matmul(
                                pv0[:], V[pkb][:, h, :], prev_pt[:, 0:512],
                                start=(pkb == 0), stop=False)
                            nc.tensor.matmul(
                                pv1[:], V[pkb][:, h, :], prev_pt[:, 512:1024],
                                start=(pkb == 0), stop=False)
                        prev_pt = pt
                    nc.tensor.matmul(pv0[:], V[N_KB - 1][:, h, :],
                                     prev_pt[:, 0:512],
                                     start=False, stop=True)
                    nc.tensor.matmul(pv1[:], V[N_KB - 1][:, h, :],
                                     prev_pt[:, 512:1024],
                                     start=False, stop=True)

                    for half, pv in ((0, pv0), (1, pv1)):
                        qs = slice((2 * qp + half) * 512,
                                   (2 * qp + half + 1) * 512)
                        den = smallp.tile([1, 512], F32, tag="den",
                                          name="den")
                        nc.vector.tensor_copy(den[:], pv[DH:DH + 1, :])
                        denb = smallp.tile([64, 512], F32, tag="denb",
                                           name="denb")
                        nc.gpsimd.partition_broadcast(denb[:], den[:])
                        rec = smallp.tile([64, 512], F32, tag="rec",
                                          name="rec")
                        nc.vector.reciprocal_approx_fast(rec[:], denb[:])
                        nc.vector.tensor_tensor(
                            CT[hp][rows, qs], pv[0:DH, :], rec[:], MULT)

                # output projection + chunked ReduceScatter
                for qb in (2 * qp, 2 * qp + 1):
                    for sb in range(4 * qb, 4 * qb + 4):
                        ys = scps.tile([128, 1024], F32, tag="sc", name="ys")
                        for eo in range(2):
                            for c in range(4):
                                nc.tensor.matmul(
                                    ys[:, eo * 512:(eo + 1) * 512],
                                    CT[c][:, sb * 128:(sb + 1) * 128],
                                    woT_t[c][:, eo * 512:(eo + 1) * 512],
                                    start=(c == 0), stop=(c == 3))
                        yt = youtp.tile([128, E], F32, tag="yt", name="yt")
                        nc.vector.tensor_add(yt[:], ys[:], bob[:])
                        nc.sync.dma_start(
                            y_part[sb * 128:(sb + 1) * 128, :], yt[:])
                    for i, (r0, n, after) in enumerate(RS_CHUNKS):
                        if after != qb:
                            continue
                        nc.gpsimd.collective_compute(
                            "ReduceScatter", mybir.AluOpType.add,
                            replica_groups=[[0, 1], [2, 3], [4, 5], [6, 7]],
                            ins=[y_part[r0:r0 + n, :]],
                            outs=[y_chunks[i][:]])
                        nc.sync.dma_start(
                            y_out[r0 // 2:(r0 + n) // 2, :], y_chunks[i][:])


def _get_nc():
    if "nc" not in _CACHE:
        _CACHE["nc"] = _build()
    return _CACHE["nc"]


def _make_in_maps(x, wq, wk, wv, wo, bo):
    bf16 = ml_dtypes.bfloat16
    x, wq, wk, wv, wo, bo = (np.asarray(a) for a in (x, wq, wk, wv, wo, bo))
    in_maps = []
    for c in range(N_CORES):
        b, g = c // TP, c % TP
        h0 = g * H_LOC
        xT_l = np.ascontiguousarray(x[b].T).astype(bf16)
        wq_l = np.ascontiguousarray(
            wq[h0:h0 + H_LOC].transpose(1, 0, 2).reshape(E, EI_LOC)).astype(bf16)
        wk_l = np.ascontiguousarray(
            wk[h0:h0 + H_LOC].transpose(1, 0, 2).reshape(E, EI_LOC)).astype(bf16)
        wv_l = np.ascontiguousarray(
            wv[h0:h0 + H_LOC].transpose(1, 0, 2).reshape(E, EI_LOC)).astype(bf16)
        woT_l = np.ascontiguousarray(
            wo[:, g * EI_LOC:(g + 1) * EI_LOC].T).astype(bf16)
        bob = np.broadcast_to(bo.astype(np.float32) / TP, (128, E)).copy()
        in_maps.append({
            "xT": xT_l, "wq": wq_l, "wk": wk_l, "wv": wv_l, "woT": woT_l,
            "bob": bob,
        })
    return in_maps


def _assemble(results):
    out = np.empty((B, S, E), dtype=np.float32)
    for c in range(N_CORES):
        b, g = c // TP, c % TP
        y = results[c]["y"]
        for r0, n, _ in RS_CHUNKS:
            half = n // 2
            out[b, r0 + g * half:r0 + (g + 1) * half, :] =                 y[r0 // 2:r0 // 2 + half, :]
    return out


def kernel(x, wq, wk, wv, wo, bo):
    nc = _get_nc()
    in_maps = _make_in_maps(x, wq, wk, wv, wo, bo)
    res = run_bass_kernel_spmd(nc, in_maps, list(range(N_CORES)))
    return _assemble(res.results)
